# revision 42
# baseline (speedup 1.0000x reference)
"""PixelPrototypeDistanceLoss on 8 Trainium2 NeuronCores.

Math: for each pixel p with label lb_p != 19:
    logit_p = emb_pixel_p . segment_queue[lb_p]
    loss = mean((1 - logit_p)^2)  over valid pixels

Trick: with onehot[c,p] = (lb_p == c) for c in [0,19), ignored pixels match
nothing, so
    sum_p valid*(1-logit)^2 = count - 2*S1 + S2
with count = sum(onehot), S1 = sum(sim*onehot), S2 = sum(sim^2*onehot),
all plain full reductions over the [C, N] similarity map -- no gather.

Sharding: batch dim across the 8 cores (one image each).  Per core:
  sim tiles [19, 512] computed as QT.T @ X with X = emb[b] reshaped [256, N]
  (already channels-first, no transpose needed).  Four pixel-blocks stacked at
  partition offsets 0/32/64/96 (PE tile_position constraint) so the DVE sees
  [128, C_g] blocks.  QT is zero-padded to 32 columns so every PSUM row is
  written (no stale reads).  scalar_tensor_tensor fuses (lbb==cls)*sim with
  the row-sum for S1; ScalarE activation(Square) accumulates S2.  Valid-count
  comes from one tensor_scalar(not_equal) over the raw labels.
Pipelining: emb is cast to fp8-e4m3 on the host (memory-bound problem; PE
matmul runs at 1 cyc/row for fp8 so it keeps pace), all input tiles are
resident (no buffer reuse) and their DMAs are issued upfront, split across
the two HWDGE queues (sync + scalar) to parallelize descriptor generation.
The last two groups are half-sized to shorten the serial drain at the tail.
Host: sums the tiny per-core partial accumulators in f64.
"""

import numpy as np
import ml_dtypes

import concourse.bacc as bacc
import concourse.mybir as mybir
from concourse.tile import TileContext
from concourse import bass_utils

# Problem dims (hardcoded per harness contract).
B, D, H, W, C = 8, 256, 128, 128, 19
NPX = H * W          # 16384 pixels per core (one batch image)
NCORES = 8
IGNORE = 19.0

CP = 32              # padded class count (PE tile_position granularity)
F = 512              # matmul moving dim (one PSUM bank of f32)
# pixel groups; each is split into 4 class-stacks of C_g = n/4 psum columns.
# small first group -> early pipeline start; tapered tail -> short drain
GROUPS = [2048, 4096, 4096, 4096, 1024, 1024]
assert sum(GROUPS) == NPX

EMB_DT = mybir.dt.float8e4
EMB_NP = ml_dtypes.float8_e4m3
LB_DT = mybir.dt.uint8
LB_NP = np.uint8

NG = len(GROUPS)
LBB_COLS = NPX // 4

_CACHE = {}


def _build():
    if "nc" in _CACHE:
        return _CACHE["nc"]
    nc = bacc.Bacc(
        "TRN2",
        target_bir_lowering=False,
        debug=False,
        enable_asserts=False,
    )
    # x packed on host as [128, 2*NPX]: group g's block at cols
    # [2*base_g, 2*base_g + 2*n), chunk k at block-local cols [k*n, (k+1)*n)
    x_t = nc.dram_tensor("x", [128, 2 * NPX], EMB_DT, kind="ExternalInput")
    lb_t = nc.dram_tensor("lb", [128, 128], mybir.dt.float32, kind="ExternalInput")
    lbb_t = nc.dram_tensor("lbb", [128, LBB_COLS], LB_DT, kind="ExternalInput")
    qt_t = nc.dram_tensor("qt", [128, 2 * CP], EMB_DT, kind="ExternalInput")
    out_t = nc.dram_tensor("out", [128, 1 + 2 * NG], mybir.dt.float32,
                           kind="ExternalOutput")

    x = x_t.ap()
    lb = lb_t.ap()
    lbb = lbb_t.ap()
    qt = qt_t.ap()
    out = out_t.ap()

    AO = mybir.AluOpType

    with TileContext(nc) as tc:
        with (
            tc.tile_pool(name="const", bufs=1) as cpool,
            tc.tile_pool(name="xp", bufs=1) as xpool,
            tc.tile_pool(name="lbp", bufs=1) as lbpool,
            tc.tile_pool(name="scr", bufs=3) as spool,
            tc.tile_pool(name="acc", bufs=1) as apool,
            tc.tile_pool(name="ps", bufs=4, space="PSUM") as pspool,
        ):
            # all input tiles are resident; issue every DMA upfront on ONE
            # HWDGE queue (two queues contend for a shared cap and starve
            # each other; a single queue sustains ~350 GB/s).  Big-row
            # transfers lead the queue; tiny-row transfers go last.
            qt_sb = cpool.tile([128, 2 * CP], EMB_DT)
            xt = {}
            base = 0
            for g, n in enumerate(GROUPS):
                t = xpool.tile([128, 2 * n], EMB_DT, tag=f"xg{g}")
                nc.sync.dma_start(t[:, :], x[:, 2 * base:2 * base + 2 * n])
                xt[g] = t
                if g == 0:
                    # qt packed on host as [128, 2*CP]: col 32k+c = QT[128k+p, c]
                    nc.sync.dma_start(qt_sb[:, :], qt[:, :])
                    # one tile for all groups' host-built onehot (uint8 0/1)
                    lbbt = lbpool.tile([128, LBB_COLS], LB_DT)
                    nc.sync.dma_start(lbbt[:, :], lbb[:, :])
                base += n
            lb_sb = cpool.tile([128, 128], mybir.dt.float32)
            nc.sync.dma_start(lb_sb[:, :], lb[:, :])

            cnt_s1 = apool.tile([128, 1 + NG], mybir.dt.float32)
            s2 = apool.tile([128, NG], mybir.dt.float32)
            junk = apool.tile([128, 128], mybir.dt.float32)

            # count of valid pixels (per partition; host sums).
            # op1 is the reduce op when accum_out is given.
            nc.vector.tensor_scalar(junk[:, :], lb_sb[:, :], IGNORE, None,
                                    AO.not_equal, AO.add,
                                    accum_out=cnt_s1[:, 0:1])

            off = 0
            for g, n in enumerate(GROUPS):
                cg = n // 4
                ps = pspool.tile([128, cg], mybir.dt.float32, tag="ps")
                fb = min(F, cg)  # matmul moving-dim block
                for s in range(4):
                    for m in range(cg // fb):
                        for k in range(2):
                            nc.tensor.matmul(
                                out=ps[CP * s:CP * (s + 1),
                                       m * fb:(m + 1) * fb],
                                lhsT=qt_sb[:, k * CP:(k + 1) * CP],
                                rhs=xt[g][:, k * n + s * cg + m * fb:
                                          k * n + s * cg + (m + 1) * fb],
                                start=(k == 0), stop=(k == 1),
                                tile_position=(0, CP * s))

                t1 = spool.tile([128, cg], mybir.dt.float32, tag="t1")
                t2 = spool.tile([128, cg], mybir.dt.float32, tag="t2")
                # t1 = onehot * sim ; s1[:, g] = row-sum(t1)
                nc.vector.scalar_tensor_tensor(
                    out=t1[:, :], in0=lbbt[:, off:off + cg], scalar=1.0,
                    in1=ps[:, :], op0=AO.mult, op1=AO.mult,
                    accum_out=cnt_s1[:, 1 + g:2 + g])
                # t2 = t1^2 = onehot*sim^2 ; s2[:, g] = row-sum(t2)
                # on the otherwise-idle scalar engine
                nc.scalar.activation(
                    t2[:, :], t1[:, :], mybir.ActivationFunctionType.Square,
                    accum_out=s2[:, g:g + 1])
                off += cg

            nc.sync.dma_start(out[:, 0:1 + NG], cnt_s1[:, :])
            nc.sync.dma_start(out[:, 1 + NG:1 + 2 * NG], s2[:, :])

    nc.compile()
    _CACHE["nc"] = nc
    return nc


def _prep_in_maps(emb, lb, segment_queue):
    emb = np.asarray(emb)
    lb = np.asarray(lb)
    q = np.asarray(segment_queue, dtype=np.float32)

    qt = np.zeros((D, CP), np.float32)
    qt[:, :C] = q.T
    # pack [2,128,CP] -> [128, 2*CP]: col 32k+c = QT[128k+p, c]
    qt = np.ascontiguousarray(
        qt.reshape(2, 128, CP).transpose(1, 0, 2).reshape(128, 2 * CP)
        .astype(EMB_NP))

    cls_pat = np.where(np.arange(CP) < C, np.arange(CP), -1)  # [32]

    in_maps = []
    for b in range(B):
        x8 = emb[b].reshape(2, 128, NPX).astype(EMB_NP)
        # pack per group: xb[p, 2*base + k*n + j] = x8[k, p, base + j]
        xb = np.empty((128, 2 * NPX), EMB_NP)
        base = 0
        for n in GROUPS:
            blk = x8[:, :, base:base + n]            # [2, 128, n]
            xb[:, 2 * base:2 * base + 2 * n] = (
                blk.transpose(1, 0, 2).reshape(128, 2 * n))
            base += n
        lbf = lb[b].reshape(-1).astype(np.float32)
        # onehot[32*s + c, off_g + j] = (lb[base_g + s*C_g + j] == c)
        segs = []
        base = 0
        for n in GROUPS:
            cg = n // 4
            seg = lbf[base:base + n].reshape(4, 1, cg)
            segs.append((seg == cls_pat[None, :, None]).reshape(128, cg))
            base += n
        lbb = np.ascontiguousarray(
            np.concatenate(segs, axis=1).astype(LB_NP))
        in_maps.append({
            "x": xb,
            "lb": np.ascontiguousarray(lbf.reshape(128, 128)),
            "lbb": lbb,
            "qt": qt,
        })
    return in_maps


def _reduce_outputs(results):
    cnt = 0.0
    s1 = 0.0
    s2 = 0.0
    for r in results:
        o = np.asarray(r["out"], dtype=np.float64)
        cnt += o[:, 0].sum()
        s1 += o[:, 1:1 + NG].sum()
        s2 += o[:, 1 + NG:1 + 2 * NG].sum()
    num = cnt - 2.0 * s1 + s2
    return np.float32(num / cnt)


def _partials(out_arr):
    o = np.asarray(out_arr, dtype=np.float64)
    return o[:, 0].sum(), o[:, 1:1 + NG].sum(), o[:, 1 + NG:].sum()


def run_on_cores(inputs, **kwargs):
    """Run the bass kernel on cores 0-7; returns (loss, BassKernelResults).

    The device occasionally reports a transient NRT_EXEC_UNIT_UNRECOVERABLE
    on a run that succeeds on immediate retry; retry a couple of times.
    """
    nc = _build()
    in_maps = _prep_in_maps(**inputs)
    last_err = None
    for _ in range(3):
        try:
            res = bass_utils.run_bass_kernel_spmd(
                nc, in_maps, core_ids=list(range(NCORES)), **kwargs)
            return _reduce_outputs(res.results), res
        except Exception as e:  # transient device wedge -> retry
            last_err = e
    raise last_err


def kernel(emb, lb, segment_queue):
    loss, _ = run_on_cores({"emb": emb, "lb": lb, "segment_queue": segment_queue})
    return loss
